# revision 35
# baseline (speedup 1.0000x reference)
"""Trainium2 Bass kernel for MessageControlGraphAttentionLayer.

Shapes (hardcoded): x (4,256,256) f32, boundary (4,256) int32,
att_proj_w (256,256), att_proj_b (256,), att_weight (256,8),
proj_att_w (2048,256), proj_att_b (256,), proj_no_w (256,256),
proj_no_b (256,), bn_gamma (256,), bn_beta (256,).

Sharding: 8 cores, core c handles batch b=c//2, query rows
j in [128*(c%2), 128*(c%2)+128). All weights replicated. BN batch
stats are all-reduced across the 8 cores with a device collective.

Math (per core, J=128 query rows, T=256 keys, D=O=256, H=8):
  rhs_j = xT * xT[:,j] per-partition scale (DVE, fp8e4m3 out)
  mm1: s_j[o,k] = sum_d W1[d,o] * rhs_j[d,k] -- fp8 DoubleRow matmul
       (lhsT [128,2dc,128], rhs [128,2dc,256]) = 0.5 cyc/row on PE.
  tanh: one merged ACT instr per 2 j's over [128,1024] psum -> fp8 a.
       (att_proj_b==0 lets the bias be dropped; a with-bias fallback
       splits per-oc and uses the ACT bias operand.)
  mm2: att[(j,h),k] += W2dr[slot].T @ a_j -- fp8 DoubleRow, W2 embedded
       in zero-padded (128,2,128) tiles so 16 j's * 8 heads pack densely
       into 128 psum partitions per block.
  mask-mul (Pool) + exp (accum row sums) + 1/Z scale -> attention
  PE-transpose (jh,k)->(k,jh); mm3: x1T[d,(j,h)] = xk.T @ enT (f32r)
  mm4 (bf16): y[o,j] = sum_h Wph[h].T @ x1T[:,:,h] + Wn.T @ xTb (+bias)
  BN stats (sum, sumsq) -> AllReduce over 8 cores -> affine + selu.

fp8e4m3 on mm1/mm2 + bf16 on mm4 keeps end-to-end absmax rel err
~6e-3 (measured vs the fp32 reference) against the 2e-2 gate.
"""

import sys

if "/opt/trn_rl_repo" not in sys.path:
    sys.path.insert(0, "/opt/trn_rl_repo")

import numpy as np
import ml_dtypes

B, T, D, O, H = 4, 256, 256, 256, 8
P = 128
NCORES = 8
J = 128  # query rows per core
NBLK = 8  # blocks of 16 j per core
BN_EPS = 1e-5
SELU_LAM = 1.0507009873554805
SELU_ALPHA = 1.6732632423543772

E4NP = ml_dtypes.float8_e4m3fn
BFNP = ml_dtypes.bfloat16

_CACHE = {}
_CACHE_ETP = [None]


def _message_control_mask_np(boundary):
    Bb, Tt = boundary.shape
    s = np.cumsum(boundary.astype(np.int64), axis=1)
    spad = np.concatenate([np.zeros((Bb, 1), np.int64), s], axis=1)  # (B,T+1)
    idx = np.arange(Tt)
    jj, kk = np.meshgrid(idx, idx, indexing="ij")
    hi = np.maximum(jj, kk)
    lo = np.minimum(jj, kk)
    rng_sum = spad[:, hi + 1] - spad[:, lo]  # (B,T,T)
    mask = rng_sum == 0
    mask = mask | np.eye(Tt, dtype=bool)[None]
    return mask.astype(np.float32)


def _build_module(with_collective=True, reps=1, with_bias=False):
    from concourse import bacc, bass, tile
    import concourse.mybir as mybir
    from concourse.masks import make_identity

    f32 = mybir.dt.float32
    f32r = mybir.dt.float32r
    f8 = mybir.dt.float8e4
    bf16 = mybir.dt.bfloat16
    AF = mybir.ActivationFunctionType
    ALU = mybir.AluOpType
    DR = mybir.MatmulPerfMode.DoubleRow

    nc = bacc.Bacc("TRN2", target_bir_lowering=False, debug=False,
                   num_devices=NCORES)

    xT_d = nc.dram_tensor("xT", [D, T], f32, kind="ExternalInput")
    xk_d = nc.dram_tensor("xk", [T, D], bf16, kind="ExternalInput")
    w1dr_d = nc.dram_tensor("w1dr", [P, 2, 2, P], f8, kind="ExternalInput")
    w2dr_d = nc.dram_tensor("w2dr", [P, 16, 2, P], f8, kind="ExternalInput")
    wph_d = nc.dram_tensor("wph", [H, 2, P, O], bf16, kind="ExternalInput")
    wn_d = nc.dram_tensor("wn", [D, O], bf16, kind="ExternalInput")
    xtb_d = nc.dram_tensor("xtb", [P, 2, J], bf16, kind="ExternalInput")
    maskx_d = nc.dram_tensor("maskx", [P, NBLK, T], f32, kind="ExternalInput")
    pvec_d = nc.dram_tensor("pvec", [P, 8], f32, kind="ExternalInput")
    yout_d = nc.dram_tensor("yout", [2, P, J], f32, kind="ExternalOutput")

    with tile.TileContext(nc) as tc:
        with (
            tc.tile_pool(name="const", bufs=1) as cpool,
            tc.tile_pool(name="dram", bufs=1, space="DRAM") as dpool,
        ):
            # Tiny dummy Tanh first: forces the ACT table load (a TDRAM DMA)
            # to be queued before the multi-MB const loads, so the first real
            # tanh isn't gated ~10us on DMA traffic.
            warm = cpool.tile([P, 1], f32)
            nc.gpsimd.memset(warm[:], 0.0)
            nc.scalar.activation(warm[:], warm[:],
                                 mybir.ActivationFunctionType.Tanh)
            # w1dr + xT first: the g0 mm1 (and so the whole tanh stream)
            # waits only on these loads; HWDGE serves DMAs in order.
            w1dr_sb = cpool.tile([P, 2, 2, P], f8)
            nc.sync.dma_start(w1dr_sb[:], w1dr_d[:])
            xT_sb = cpool.tile([P, 2, T], f32)
            xT_r = xT_d.ap().rearrange("(c p) k -> p c k", p=P)
            nc.sync.dma_start(xT_sb[:, 0, :], xT_r[:, 0, :])
            nc.sync.dma_start(xT_sb[:, 1, :], xT_r[:, 1, :])
            w2dr_sb = cpool.tile([P, 16, 2, P], f8)
            nc.sync.dma_start(w2dr_sb[:], w2dr_d[:])
            maskx_sb = cpool.tile([P, NBLK, T], f32)
            nc.sync.dma_start(maskx_sb[:], maskx_d[:])
            xk_sb = cpool.tile([P, 2, D], bf16)
            nc.sync.dma_start(xk_sb[:], xk_d.ap().rearrange("(c p) d -> p c d", p=P))
            pvec_sb = cpool.tile([P, 8], f32)
            nc.sync.dma_start(pvec_sb[:], pvec_d[:])
            xtb_sb = cpool.tile([P, 2, J], bf16)
            nc.sync.dma_start(xtb_sb[:], xtb_d[:])
            wn_sb = cpool.tile([P, 2, O], bf16)
            nc.sync.dma_start(wn_sb[:], wn_d.ap().rearrange("(c p) o -> p c o", p=P))
            # wph is only needed by phase 3 -- load it last
            wph_sb = cpool.tile([P, 16, O], bf16)
            nc.sync.dma_start(wph_sb[:], wph_d.ap().rearrange("h c p o -> p (h c) o"))
            ident = cpool.tile([P, P], f32)
            make_identity(nc, ident[:])
            identb = cpool.tile([P, P], bf16)
            nc.vector.tensor_copy(identb[:], ident[:])
            x1T = cpool.tile([P, 2, J, H], bf16)  # [p][md][j][h]

            with (
                tc.tile_pool(name="work", bufs=1) as wpool,
                tc.tile_pool(name="pp1", bufs=2, space="PSUM") as pp1,
                tc.tile_pool(name="pp4", bufs=1, space="PSUM") as pp4,
                tc.tile_pool(name="pp2", bufs=1, space="PSUM") as pp2,
                tc.tile_pool(name="ppt", bufs=1, space="PSUM") as ppt,
                tc.tile_pool(name="pp3", bufs=1, space="PSUM") as pp3,
            ):
                # Host rolls the key axis by -j0 per core, so each core's
                # query columns are always 0..127 of xT (SPMD: one program).
                for _rep in range(reps):
                    # One psum tile holds both oc halves of y so mm4 chunks
                    # can accumulate across the whole block loop.
                    ps4 = pp4.tile([P, 2, J], f32, tag="p4", name="ps4")
                    # PE work whose results are needed blocks later (mm4
                    # chunks, mirror transposes) is queued as thunks and
                    # drip-fed between g-groups, so the in-order PE never
                    # delays the next urgent mm1 by more than ~100ns.
                    mm4_pending = []
                    for blk in range(NBLK):
                        psum2 = pp2.tile([P, T], f32, tag="p2", name=f"p2_{blk}")
                        for gg in range(8):
                            g = blk * 8 + gg
                            ps1 = pp1.tile([P, 2, 2, T], f32, tag="p1",
                                           name=f"p1_{g}")  # [p][oc][jj][k]
                            rhs = wpool.tile([P, 2, 2, T], f8, tag="rhs", bufs=4,
                                             name=f"rhs_{g}")  # [p][jj][dc][k]
                            for jj in range(2):
                                jl = g * 2 + jj  # local query index 0..127
                                for dc in range(2):
                                    # Pool matches DVE's pace at a 1:3 split
                                    # (one ~555ns Pool op vs three ~190ns DVE
                                    # ops per g), halving DVE rhs load.
                                    use_pool = (jj == 1) and (dc == 1)
                                    eng = nc.gpsimd if use_pool else nc.vector
                                    eng.tensor_scalar_mul(
                                        out=rhs[:, jj, dc, :],
                                        in0=xT_sb[:, dc, :],
                                        scalar1=xT_sb[:, dc, jl:jl + 1],
                                    )
                            for oc in range(2):
                                for jj in range(2):
                                    nc.tensor.matmul(
                                        ps1[:, oc, jj, :],
                                        w1dr_sb[:, oc],
                                        rhs[:, jj],
                                        start=True, stop=True,
                                        perf_mode=DR,
                                    )
                            a_t = wpool.tile([P, 2, 2, T], f8, tag="a", bufs=4,
                                             name=f"a_{g}")  # [p][oc][jj][k]
                            if with_bias:
                                for oc in range(2):
                                    nc.scalar.activation(
                                        a_t[:, oc], ps1[:, oc], AF.Tanh,
                                        bias=pvec_sb[:, oc:oc + 1])
                            else:
                                nc.scalar.activation(a_t[:], ps1[:], AF.Tanh)
                            for jj in range(2):
                                jl_blk = gg * 2 + jj  # 0..15 within block
                                nc.tensor.matmul(
                                    psum2[:],
                                    w2dr_sb[:, jl_blk],
                                    a_t[:, :, jj, :],
                                    start=(gg == 0 and jj == 0),
                                    stop=(gg == 7 and jj == 1),
                                    perf_mode=DR,
                                )
                            for _ in range(6):
                                if mm4_pending:
                                    mm4_pending.pop(0)[1]()
                        # --- block tail: mask, exp, normalize, transpose, mm3 ---
                        attm = wpool.tile([P, T], f32, tag="attm", bufs=3,
                                          name=f"attm_{blk}")
                        nc.vector.tensor_mul(attm[:], psum2[:], maskx_sb[:, blk, :])
                        e_t = wpool.tile([P, T], f32, tag="e", bufs=3,
                                         name=f"e_{blk}")
                        zsum = wpool.tile([P, 1], f32, tag="zs", bufs=2,
                                          name=f"zs_{blk}")
                        nc.scalar.activation(e_t[:], attm[:], AF.Exp,
                                             accum_out=zsum[:])
                        zinv = wpool.tile([P, 1], f32, tag="zi", bufs=2,
                                          name=f"zi_{blk}")
                        nc.vector.reciprocal(zinv[:], zsum[:])
                        en = wpool.tile([P, T], bf16, tag="en", bufs=3,
                                        name=f"en_{blk}")
                        nc.vector.tensor_scalar_mul(out=en[:], in0=e_t[:],
                                                    scalar1=zinv[:])
                        if blk % 2 == 0:
                            # [p][kc][b01][j] -- both transposes of both pair
                            # members, one copy per block
                            eT2 = wpool.tile([P, 2, 2, P], bf16, tag="eT",
                                             bufs=3, name=f"eT_{blk}")
                            _CACHE_ETP[0] = eT2
                        else:
                            eT2 = _CACHE_ETP[0]
                        # both kc-transposes land in ONE psum tile (disjoint
                        # regions), so neither waits on a DVE copy.
                        psT = ppt.tile([P, 2, P], bf16, tag="pt",
                                       name=f"psT_{blk}")
                        for kc in range(2):
                            nc.tensor.transpose(psT[:, kc, :],
                                                en[:, kc * P:(kc + 1) * P],
                                                identb[:])
                        nc.vector.tensor_copy(eT2[:, :, blk % 2, :], psT[:])
                        if blk % 2 == 1:
                            pair = blk // 2
                            ps3 = pp3.tile([P, 2, 2, P], f32, tag="p3",
                                           name=f"ps3_{blk}")  # [p][md][b01][j]
                            for md in range(2):
                                for kc in range(2):
                                    nc.tensor.matmul(
                                        ps3[:, md],
                                        xk_sb[:, kc, md * P:(md + 1) * P],
                                        eT2[:, kc],
                                        start=(kc == 0),
                                        stop=(kc == 1),
                                    )
                            nc.vector.tensor_copy(
                                x1T[:, :, pair * 32:(pair + 1) * 32, :],
                                ps3[:].rearrange("p m a (b c) -> p m (a b) c",
                                                 c=H),
                            )
                            # mm4 chunk for the 32 queries this pair finished.
                            js = slice(pair * 32, (pair + 1) * 32)

                            def _mk(oc, h, dc, js, first, stop):
                                if h < H:
                                    def f():
                                        nc.tensor.matmul(
                                            ps4[:, oc, js],
                                            wph_sb[:, h * 2 + dc,
                                                   oc * P:(oc + 1) * P],
                                            x1T[:, dc, js, h],
                                            start=first, stop=stop,
                                        )
                                else:
                                    def f():
                                        nc.tensor.matmul(
                                            ps4[:, oc, js],
                                            wn_sb[:, dc, oc * P:(oc + 1) * P],
                                            xtb_sb[:, dc, js],
                                            start=first, stop=stop,
                                        )
                                return f

                            for oc in range(2):
                                for h in range(H + 1):
                                    for dc in range(2):
                                        mm4_pending.append((oc, _mk(
                                            oc, h, dc, js,
                                            first=(h == 0 and dc == 0),
                                            stop=(h == H and dc == 1))))

                    # ---------------- phase 3: y, BN stats ----------------
                    # flush leftover mm4 work (last pair's), oc-interleaved
                    # with the y/sq activations so y0 never waits on oc1.
                    y_t = []
                    stats = wpool.tile([P, 4], f32, tag="stats", name="stats")
                    for oc in range(2):
                        for foc, f in mm4_pending:
                            if foc == oc:
                                f()
                        yt = wpool.tile([P, J], f32, tag=f"y{oc}", name=f"y_{oc}")
                        nc.scalar.activation(yt[:], ps4[:, oc, :], AF.Identity,
                                             bias=pvec_sb[:, 2 + oc:3 + oc],
                                             accum_out=stats[:, oc:oc + 1])
                        y_t.append(yt)
                        # sumsq straight from psum: Square(ps4 + bias) == y^2,
                        # so sq does not wait on yt's write-ack.
                        sq = wpool.tile([P, J], f32, tag="sq", bufs=2,
                                        name=f"sq_{oc}")
                        nc.scalar.activation(sq[:], ps4[:, oc, :], AF.Square,
                                             bias=pvec_sb[:, 2 + oc:3 + oc],
                                             accum_out=stats[:, 2 + oc:3 + oc])
                    mm4_pending = []

                    # ---------------- BN all-reduce + affine + selu ----------------
                    # Issue the stats round-trip from the Pool sequencer: its
                    # DMA dispatch is ~25ns vs ~565ns on SP, and these three
                    # hops sit on the serial tail.
                    cc_in = dpool.tile([P, 4], f32, name="cc_in")
                    cc_out = dpool.tile([P, 4], f32, addr_space="Shared",
                                        name="cc_out")
                    nc.sync.dma_start(cc_in[:], stats[:])
                    if with_collective:
                        nc.gpsimd.collective_compute(
                            "AllReduce",
                            ALU.add,
                            replica_groups=[list(range(NCORES))],
                            ins=[cc_in.opt()],
                            outs=[cc_out.opt()],
                        )
                    else:  # perf-model probe only: skip the collective
                        nc.sync.dma_start(cc_out[:], cc_in[:])
                    statg = wpool.tile([P, 4], f32, tag="statg", name="statg")
                    nc.sync.dma_start(statg[:], cc_out[:])

                    NTOT = float(B * T)

                    def wt2(nm):
                        return wpool.tile([P, 2], f32, tag=nm, name=nm)

                    # statg cols: [s1_oc0, s1_oc1, s2_oc0, s2_oc1]
                    mom = wpool.tile([P, 4], f32, tag="mom", name="mom")
                    nc.vector.tensor_scalar_mul(out=mom[:, 0:2],
                                                in0=statg[:, 0:2],
                                                scalar1=1.0 / NTOT)
                    nc.vector.tensor_scalar(out=mom[:, 2:4],
                                            in0=statg[:, 2:4],
                                            scalar1=1.0 / NTOT,
                                            scalar2=BN_EPS,
                                            op0=ALU.mult, op1=ALU.add)
                    mu = mom[:, 0:2]
                    varp = mom[:, 2:4]
                    musq = wt2("musq")
                    nc.vector.tensor_mul(musq[:], mu, mu)
                    nc.vector.tensor_sub(varp, varp, musq[:])
                    # rsqrt on DVE only (no ACT table swap): quake guess + 1
                    # Newton iteration -> ~0.2% rel, far inside the gate.
                    i32 = mybir.dt.int32
                    magic = wpool.tile([P, 2], i32, tag="magic", name="magic")
                    nc.vector.memset(magic[:], 0x5F3759DF)
                    ri = wpool.tile([P, 2], i32, tag="ri", name="ri")
                    nc.vector.tensor_scalar(out=ri[:], in0=varp.bitcast(i32),
                                            scalar1=1, scalar2=None,
                                            op0=ALU.arith_shift_right)
                    rstd = wt2("rstd")
                    nc.vector.tensor_sub(rstd[:].bitcast(i32), magic[:], ri[:])
                    ra = wt2("ra")
                    rb = wt2("rb")
                    for _ in range(1):
                        nc.vector.tensor_mul(ra[:], rstd[:], rstd[:])
                        nc.vector.scalar_tensor_tensor(
                            out=rb[:], in0=ra[:], scalar=-0.5, in1=varp,
                            op0=ALU.mult, op1=ALU.mult)
                        nc.vector.tensor_scalar_add(out=rb[:], in0=rb[:],
                                                    scalar1=1.5)
                        nc.vector.tensor_mul(rstd[:], rstd[:], rb[:])
                    scl = wt2("scl")
                    nc.vector.tensor_mul(scl[:], pvec_sb[:, 4:6], rstd[:])
                    tmp = wt2("tmp")
                    nc.vector.tensor_mul(tmp[:], mu, scl[:])
                    shf = wt2("shf")
                    nc.vector.tensor_sub(shf[:], pvec_sb[:, 6:8], tmp[:])

                    z = wpool.tile([P, 2, J], f32, tag="z", name="z")
                    for oc in range(2):
                        nc.vector.tensor_scalar(out=z[:, oc, :], in0=y_t[oc][:],
                                                scalar1=scl[:, oc:oc + 1],
                                                scalar2=shf[:, oc:oc + 1],
                                                op0=ALU.mult, op1=ALU.add)
                    # selu on the merged (P, 2*J) tile
                    neg = wpool.tile([P, 2, J], f32, tag="neg", name="neg")
                    nc.vector.tensor_scalar_min(out=neg[:], in0=z[:], scalar1=0.0)
                    ep = wpool.tile([P, 2, J], f32, tag="ep", name="ep")
                    nc.scalar.activation(ep[:], neg[:], AF.Exp)
                    em = wpool.tile([P, 2, J], f32, tag="em", name="em")
                    nc.vector.tensor_scalar(
                        out=em[:], in0=ep[:],
                        scalar1=SELU_LAM * SELU_ALPHA,
                        scalar2=-SELU_LAM * SELU_ALPHA,
                        op0=ALU.mult, op1=ALU.add)
                    pos = wpool.tile([P, 2, J], f32, tag="pos", name="pos")
                    nc.vector.tensor_scalar_max(out=pos[:], in0=z[:], scalar1=0.0)
                    outz = wpool.tile([P, 2, J], f32, tag="outz", name="outz")
                    nc.vector.scalar_tensor_tensor(
                        out=outz[:], in0=pos[:], scalar=SELU_LAM, in1=em[:],
                        op0=ALU.mult, op1=ALU.add)
                    nc.sync.dma_start(yout_d.ap().rearrange("c p j -> p c j"),
                                      outz[:])

    nc.compile()
    return nc


def _prep_inputs(x, boundary, att_proj_w, att_proj_b, att_weight,
                 proj_att_w, proj_att_b, proj_no_w, proj_no_b,
                 bn_gamma, bn_beta):
    mask = _message_control_mask_np(np.asarray(boundary))
    x = np.ascontiguousarray(np.asarray(x, dtype=np.float32))
    w1 = np.asarray(att_proj_w, dtype=np.float32)
    w2 = np.asarray(att_weight, dtype=np.float32)

    # mm1 DoubleRow stationary: w1dr[p, oc, dc, m] = W1[dc*128+p, oc*128+m]
    w1dr = np.ascontiguousarray(
        w1.astype(E4NP).reshape(2, P, 2, P).transpose(1, 2, 0, 3))
    # mm2 DoubleRow sparse pack: per slot jl, W2 chunk at cols [8jl, 8jl+8)
    w2q = w2.astype(E4NP)
    w2dr = np.zeros((P, 16, 2, P), dtype=E4NP)
    for jl in range(16):
        for oc in range(2):
            w2dr[:, jl, oc, 8 * jl:8 * jl + 8] = w2q[oc * P:(oc + 1) * P, :]

    wph = np.ascontiguousarray(
        np.asarray(proj_att_w, dtype=np.float32)
        .reshape(D, H, O).transpose(1, 0, 2).reshape(H, 2, P, O)).astype(BFNP)
    wn = np.asarray(proj_no_w, dtype=np.float32).astype(BFNP)

    by = (np.asarray(proj_att_b, dtype=np.float32)
          + np.asarray(proj_no_b, dtype=np.float32))
    pvec = np.zeros((P, 8), dtype=np.float32)
    b1 = np.asarray(att_proj_b, dtype=np.float32)
    g = np.asarray(bn_gamma, dtype=np.float32)
    be = np.asarray(bn_beta, dtype=np.float32)
    for oc in range(2):
        pvec[:, oc] = b1[oc * P:(oc + 1) * P]
        pvec[:, 2 + oc] = by[oc * P:(oc + 1) * P]
        pvec[:, 4 + oc] = g[oc * P:(oc + 1) * P]
        pvec[:, 6 + oc] = be[oc * P:(oc + 1) * P]

    in_maps = []
    for c in range(NCORES):
        b = c // 2
        j0 = (c % 2) * J
        xb = x[b]  # (T, D)
        xT = np.ascontiguousarray(xb.T)  # (D, T)
        # roll keys so this core's query columns are always 0..127
        xTq = np.ascontiguousarray(np.roll(xT, -j0, axis=1))
        xkq = np.ascontiguousarray(np.roll(xb, -j0, axis=0)).astype(BFNP)
        xtb = np.ascontiguousarray(
            xTq.reshape(2, P, T)[:, :, :J].transpose(1, 0, 2)).astype(BFNP)
        m = mask[b, j0:j0 + J]  # (J, T) in original key order
        mq = np.roll(m, -j0, axis=1)
        maskx = np.ascontiguousarray(
            np.repeat(mq.reshape(NBLK, 16, 1, T), H, axis=2)
            .transpose(1, 2, 0, 3).reshape(P, NBLK, T))
        in_maps.append({
            "xT": xTq,
            "xk": xkq,
            "w1dr": w1dr,
            "w2dr": w2dr,
            "wph": wph,
            "wn": wn,
            "xtb": xtb,
            "maskx": maskx,
            "pvec": pvec,
        })
    return in_maps


def kernel(**inputs):
    from concourse.bass_utils import run_bass_kernel_spmd

    with_bias = bool(np.any(np.asarray(inputs["att_proj_b"], np.float32)))
    key = ("nc", with_bias)
    if key not in _CACHE:
        _CACHE[key] = _build_module(with_bias=with_bias)
    nc = _CACHE[key]

    in_maps = _prep_inputs(**inputs)
    res = run_bass_kernel_spmd(nc, in_maps, core_ids=list(range(NCORES)),
                               **_CACHE.get("run_kwargs", {}))
    _CACHE["last_results"] = res

    out = np.zeros((B, T, O), dtype=np.float32)
    for c in range(NCORES):
        b = c // 2
        j0 = (c % 2) * J
        yc = res.results[c]["yout"]  # (2, P, J): (oc, o_sub, j_local)
        # keys were rolled but output rows are the queries (j local order is
        # 0..127 == global j0..j0+127); columns are o (unrolled). The roll
        # only permuted the key/contraction axis, which is summed out.
        out[b, j0:j0 + J, :] = yc.reshape(O, J).T
    return out


if __name__ == "__main__":
    # smoke build
    _build_module()
    print("build ok")
